# revision 1
# baseline (speedup 1.0000x reference)
"""Euclidean distance (cdist) kernel for Trainium2, 8 NeuronCores.

out[b, j] = || x[b, :] - weight[:, j] ||_2   for x [4096, 64], weight [64, 50000].

Strategy (tensor-parallel over prototypes, per sharding hint):
  - K = 50000 is split into 8 slabs of 6250, one per core; x is replicated.
  - Host prep folds the distance decomposition into a single matmul:
        xa[b, :]  = [x[b, :], -0.5]          (augmented column)
        wa[:, j]  = [weight[:, j], w2[j]]    (augmented row, w2 = sum_d w^2)
    so   (xa @ wa)[b, j] = xw[b, j] - 0.5 * w2[j]
    and  dist[b, j] = sqrt(-2 * (xa@wa)[b, j] + x2[b])
    which is one PE matmul (contraction 65) + one ScalarE Sqrt activation with
    per-partition bias x2 and scale -2.
  - Per core: 32 b-tiles of 128 rows; per b-tile 13 matmuls of <=512 cols into
    PSUM, activation PSUM->SBUF, then a single contiguous 3.2 MB DMA store.
"""

import numpy as np
from contextlib import ExitStack

import concourse.bass as bass
import concourse.bacc as bacc
import concourse.tile as tile
from concourse import mybir
from concourse.bass_utils import run_bass_kernel_spmd

B, D, K = 4096, 64, 50000
NCORES = 8
KS = K // NCORES  # 6250 columns per core
P = 128
NBT = B // P      # 32 batch tiles
JT = 512          # matmul free-dim tile (one PSUM bank of fp32)
DA = D + 1        # augmented contraction size

F32 = mybir.dt.float32


def build_nc(b=B, ks=KS, use_f32r=False):
    nbt = b // P
    nc = bacc.Bacc("TRN2", target_bir_lowering=False, debug=False)
    xat = nc.dram_tensor("xat", [DA, b], F32, kind="ExternalInput").ap()
    wa = nc.dram_tensor("wa", [DA, ks], F32, kind="ExternalInput").ap()
    x2 = nc.dram_tensor("x2", [P, nbt], F32, kind="ExternalInput").ap()
    out = nc.dram_tensor("out", [b, ks], F32, kind="ExternalOutput").ap()

    jtiles = [(j0, min(JT, ks - j0)) for j0 in range(0, ks, JT)]

    with tile.TileContext(nc) as tc:
        with ExitStack() as ctx:
            singles = ctx.enter_context(tc.tile_pool(name="singles", bufs=1))
            outp = ctx.enter_context(tc.tile_pool(name="outp", bufs=2))
            psum = ctx.enter_context(tc.tile_pool(name="psum", bufs=8, space="PSUM"))

            wa_sb = singles.tile([DA, ks], F32)
            nc.sync.dma_start(out=wa_sb, in_=wa)
            xat_sb = singles.tile([DA, b], F32)
            nc.sync.dma_start(out=xat_sb, in_=xat)
            x2_sb = singles.tile([P, nbt], F32)
            nc.sync.dma_start(out=x2_sb, in_=x2)

            if use_f32r:
                wa_mm = wa_sb.bitcast(mybir.dt.float32r)
                xat_mm = xat_sb.bitcast(mybir.dt.float32r)
            else:
                wa_mm = wa_sb
                xat_mm = xat_sb

            for ib in range(nbt):
                ot = outp.tile([P, ks], F32)
                for (j0, jn) in jtiles:
                    pt = psum.tile([P, JT], F32)
                    nc.tensor.matmul(
                        pt[:, :jn],
                        xat_mm[:, ib * P:(ib + 1) * P],
                        wa_mm[:, j0:j0 + jn],
                        start=True,
                        stop=True,
                    )
                    nc.scalar.activation(
                        ot[:, j0:j0 + jn],
                        pt[:, :jn],
                        mybir.ActivationFunctionType.Sqrt,
                        bias=x2_sb[:, ib:ib + 1],
                        scale=-2.0,
                    )
                nc.sync.dma_start(out=out[ib * P:(ib + 1) * P, :], in_=ot)
    nc.compile()
    return nc


def prep_inputs(x, weight):
    """Host-side prep: augmented transposed x, augmented weight, x^2 tiles."""
    x = np.ascontiguousarray(x, dtype=np.float32)
    weight = np.ascontiguousarray(weight, dtype=np.float32)
    b = x.shape[0]
    x2 = (x.astype(np.float64) ** 2).sum(axis=1).astype(np.float32)     # [B]
    w2 = (weight.astype(np.float64) ** 2).sum(axis=0).astype(np.float32)  # [K]
    xat = np.empty((DA, b), dtype=np.float32)
    xat[:D, :] = x.T
    xat[D, :] = -0.5
    wa = np.empty((DA, weight.shape[1]), dtype=np.float32)
    wa[:D, :] = weight
    wa[D, :] = w2
    x2t = np.ascontiguousarray(x2.reshape(b // P, P).T)                 # [P, NBT]
    return xat, wa, x2t


_nc_cache = {}


def _get_nc():
    if "nc" not in _nc_cache:
        _nc_cache["nc"] = build_nc()
    return _nc_cache["nc"]


def kernel(x, weight):
    xat, wa, x2t = prep_inputs(x, weight)
    nc = _get_nc()
    in_maps = [
        {"xat": xat,
         "wa": np.ascontiguousarray(wa[:, i * KS:(i + 1) * KS]),
         "x2": x2t}
        for i in range(NCORES)
    ]
    res = run_bass_kernel_spmd(nc, in_maps, core_ids=list(range(NCORES)))
    return np.concatenate([res.results[i]["out"] for i in range(NCORES)], axis=1)


# revision 3
# speedup vs baseline: 2.0667x; 2.0667x over previous
"""Euclidean distance (cdist) kernel for Trainium2, 8 NeuronCores.

out[b, j] = || x[b, :] - weight[:, j] ||_2   for x [4096, 64], weight [64, 50000].

Sharding (per hint): K = 50000 split into 8 slabs of 6250, one per core
(tensor-parallel over prototypes); x replicated; no cross-core reduction.

Math: dist^2 = x2[b] + w2[j] - 2*x@w. The matmul runs in fp32r (the PE's
fast fp32 mode, RNE-rounded to 11 mantissa bits) at 4x the fp32 rate, with
full fp32-level accuracy recovered via a Dekker-style hi/lo split that
exploits the unused contraction capacity (D=64 of 128 partitions):

  mm1: lhsT=[xs_hi; xs_lo] (128 rows) rhs=[w_hi; w_hi]       -> -2x @ w_hi
  mm2: lhsT=[xs_hi; 1; 1]  (66 rows)  rhs=[w_lo; w2_hi; w2_lo]
                                              -> -2x @ w_lo + w2  (accum)
  where xs = -2x, v_hi = rne11(v), v_lo = rne11(v - v_hi).
  PSUM = -2*x'@w' + w2   with x', w' accurate to 22+ mantissa bits.
  ScalarE: out = sqrt(PSUM + x2[b])  (x2 as exact per-partition bias).

All hi/lo operands are rounded on the host (exact emulation of the HW's
fp32r RNE-11 rounding), shipped as float32r DRAM tensors.

Per core: 32 b-tiles of 128 rows; per b-tile 13 j-tiles of <=512 cols
(one PSUM bank); per b-tile a single contiguous 3.2 MB DMA store.
"""

import numpy as np
from contextlib import ExitStack

import concourse.bass as bass
import concourse.bacc as bacc
import concourse.tile as tile
from concourse import mybir
from concourse.bass_utils import run_bass_kernel_spmd

B, D, K = 4096, 64, 50000
NCORES = 8
KS = K // NCORES  # 6250 columns per core
P = 128
JT = 512          # matmul free-dim tile (one PSUM bank of fp32)
D2 = 2 * D        # 128: stacked hi/lo contraction for mm1
DL = D + 2        # 66: contraction for mm2 (w_lo + w2_hi + w2_lo rows)

F32 = mybir.dt.float32
F32R = mybir.dt.float32r


def build_nc(b=B, ks=KS):
    nbt = b // P
    nc = bacc.Bacc("TRN2", target_bir_lowering=False, debug=False)
    xs128 = nc.dram_tensor("xs128", [D2, b], F32R, kind="ExternalInput").ap()
    xs66 = nc.dram_tensor("xs66", [DL, b], F32R, kind="ExternalInput").ap()
    wst1 = nc.dram_tensor("wst1", [D2, ks], F32R, kind="ExternalInput").ap()
    wst2 = nc.dram_tensor("wst2", [DL, ks], F32R, kind="ExternalInput").ap()
    x2 = nc.dram_tensor("x2", [P, nbt], F32, kind="ExternalInput").ap()
    out = nc.dram_tensor("out", [b, ks], F32, kind="ExternalOutput").ap()

    jtiles = [(j0, min(JT, ks - j0)) for j0 in range(0, ks, JT)]

    with tile.TileContext(nc) as tc:
        with ExitStack() as ctx:
            singles = ctx.enter_context(tc.tile_pool(name="singles", bufs=1))
            outp = ctx.enter_context(tc.tile_pool(name="outp", bufs=2))
            psum = ctx.enter_context(tc.tile_pool(name="psum", bufs=8, space="PSUM"))

            wst1_sb = singles.tile([D2, ks], F32R)
            nc.sync.dma_start(out=wst1_sb, in_=wst1)
            wst2_sb = singles.tile([DL, ks], F32R)
            nc.sync.dma_start(out=wst2_sb, in_=wst2)
            xs128_sb = singles.tile([D2, b], F32R)
            nc.sync.dma_start(out=xs128_sb, in_=xs128)
            xs66_sb = singles.tile([DL, b], F32R)
            nc.sync.dma_start(out=xs66_sb, in_=xs66)
            x2_sb = singles.tile([P, nbt], F32)
            nc.sync.dma_start(out=x2_sb, in_=x2)

            for ib in range(nbt):
                ot = outp.tile([P, ks], F32)
                for (j0, jn) in jtiles:
                    pt = psum.tile([P, JT], F32)
                    nc.tensor.matmul(
                        pt[:, :jn],
                        xs128_sb[:, ib * P:(ib + 1) * P],
                        wst1_sb[:, j0:j0 + jn],
                        start=True,
                        stop=False,
                    )
                    nc.tensor.matmul(
                        pt[:, :jn],
                        xs66_sb[:, ib * P:(ib + 1) * P],
                        wst2_sb[:, j0:j0 + jn],
                        start=False,
                        stop=True,
                    )
                    nc.scalar.activation(
                        ot[:, j0:j0 + jn],
                        pt[:, :jn],
                        mybir.ActivationFunctionType.Sqrt,
                        bias=x2_sb[:, ib:ib + 1],
                        scale=1.0,
                    )
                nc.sync.dma_start(out=out[ib * P:(ib + 1) * P, :], in_=ot)
    nc.compile()
    return nc


def _rne11(x):
    """HW-exact fp32r rounding: RNE to 11 mantissa bits."""
    x = np.asarray(x, np.float32)
    u = x.view(np.uint32).astype(np.uint64)
    shift = np.uint64(12)
    half = np.uint64(1 << 11)
    lsb = (u >> shift) & np.uint64(1)
    u2 = (u + half - np.uint64(1) + lsb) >> shift << shift
    return u2.astype(np.uint32).view(np.float32)


def prep_inputs(x, weight):
    """Host-side prep: hi/lo fp32r splits and stacked operand matrices."""
    x = np.ascontiguousarray(x, dtype=np.float32)
    weight = np.ascontiguousarray(weight, dtype=np.float32)
    b, d = x.shape
    k = weight.shape[1]
    x2 = (x.astype(np.float64) ** 2).sum(axis=1).astype(np.float32)
    w2 = (weight.astype(np.float64) ** 2).sum(axis=0).astype(np.float32)

    xs = (-2.0 * x).astype(np.float32)
    xs_hi = _rne11(xs)
    xs_lo = _rne11((xs - xs_hi).astype(np.float32))
    w_hi = _rne11(weight)
    w_lo = _rne11((weight - w_hi).astype(np.float32))
    w2_hi = _rne11(w2)
    w2_lo = _rne11((w2 - w2_hi).astype(np.float32))

    xs128 = np.empty((D2, b), dtype=np.float32)
    xs128[:d] = xs_hi.T
    xs128[d:] = xs_lo.T
    xs66 = np.empty((DL, b), dtype=np.float32)
    xs66[:d] = xs_hi.T
    xs66[d:] = 1.0
    wst1 = np.empty((D2, k), dtype=np.float32)
    wst1[:d] = w_hi
    wst1[d:] = w_hi
    wst2 = np.empty((DL, k), dtype=np.float32)
    wst2[:d] = w_lo
    wst2[d] = w2_hi
    wst2[d + 1] = w2_lo
    x2t = np.ascontiguousarray(x2.reshape(b // P, P).T)  # [P, NBT]
    return xs128, xs66, wst1, wst2, x2t


_nc_cache = {}


def _get_nc():
    if "nc" not in _nc_cache:
        _nc_cache["nc"] = build_nc()
    return _nc_cache["nc"]


def make_in_maps(x, weight, ks=KS):
    xs128, xs66, wst1, wst2, x2t = prep_inputs(x, weight)
    return [
        {"xs128": xs128,
         "xs66": xs66,
         "wst1": np.ascontiguousarray(wst1[:, i * ks:(i + 1) * ks]),
         "wst2": np.ascontiguousarray(wst2[:, i * ks:(i + 1) * ks]),
         "x2": x2t}
        for i in range(NCORES)
    ]


def kernel(x, weight):
    nc = _get_nc()
    in_maps = make_in_maps(x, weight)
    res = run_bass_kernel_spmd(nc, in_maps, core_ids=list(range(NCORES)))
    return np.concatenate([res.results[i]["out"] for i in range(NCORES)], axis=1)
